# revision 11
# baseline (speedup 1.0000x reference)
"""Trainium2 Bass kernel for sparse_attention (nn_CM_Module_39934605919003).

Per-sample computation (data-parallel over batch, 1 sample per NeuronCore):
  - bilinear-resize (256->64) + threshold masks
  - gs[f] = masked cosine-ish score = sum_{c,hw} vmap*c0*crest / (vsum*C)
  - per-pixel masked softmax over f of gs
  - c_out = sum_f c_match * crest ; c_mask = 1 - sum_f c_match
  - outputs: concat([c0, c_out, c_mask], ch-axis), c_mask

Layout: pixel (h,w) -> partition p = h*2 + w//32, free col j = w%32.
CF sbuf tile: [128p, f=8, c=128, w32=32].
"""

import numpy as np
from contextlib import ExitStack

import concourse.bass as bass
import concourse.bacc as bacc
import concourse.tile as tile
import concourse.mybir as mybir
import concourse.bass_isa as bass_isa
from concourse.alu_op_type import AluOpType
from concourse.bass_utils import run_bass_kernel_spmd

F32 = mybir.dt.float32
EPS = 1e-4
C = 128          # channels
F = 8            # frames
NF = F - 1       # ref frames
H = W = 64
NJ = 32          # w32 columns per partition
P = 128          # partitions

_cache = {}


def _emit(tc, ctx, cf, vt, va, out0, out1):
    nc = tc.nc

    main = ctx.enter_context(tc.tile_pool(name="main", bufs=1))
    jv = ctx.enter_context(tc.tile_pool(name="jv", bufs=4))     # DVE junk
    jg = ctx.enter_context(tc.tile_pool(name="jg", bufs=4))     # gpsimd junk
    js = ctx.enter_context(tc.tile_pool(name="js", bufs=4))     # small junk

    CF = main.tile([P, F, C, NJ], F32, tag="CF")
    MRAW = main.tile([P, F, 2, 128], F32, tag="MRAW")
    VR = main.tile([P, F, NJ], F32, tag="VR")       # frame 0 = v_t resized
    VMAP = main.tile([P, NF, NJ], F32, tag="VMAP")
    D = main.tile([P, NF, NJ], F32, tag="D")
    GN = main.tile([P, 16], F32, tag="GN")
    GA = main.tile([P, 16], F32, tag="GA")
    GS7 = main.tile([P, 7], F32, tag="GS7")
    T7a = main.tile([P, 7], F32, tag="T7a")
    T7b = main.tile([P, 7], F32, tag="T7b")
    MV = main.tile([P, NF, NJ], F32, tag="MV")
    EX = main.tile([P, NF, NJ], F32, tag="EX")
    WT = main.tile([P, NF, NJ], F32, tag="WT")
    MX = main.tile([P, NJ], F32, tag="MX")
    SM = main.tile([P, NJ], F32, tag="SM")
    T32 = main.tile([P, NJ], F32, tag="T32")
    CM = main.tile([P, NJ], F32, tag="CM")
    ACC = main.tile([P, C, NJ], F32, tag="ACC")

    # ---- mask DMAs (rows 4i+1, 4i+2 of each 256x256 frame) ----
    # pixel (h, w) -> partition p = (w//32)*64 + h, free col j = w % 32
    vt3 = vt.rearrange("(h r4) w -> h r4 w", r4=4)
    va4 = va.rearrange("k (h r4) w -> k h r4 w", r4=4)
    for wh in range(2):
        pp = slice(wh * 64, (wh + 1) * 64)
        nc.sync.dma_start(
            out=MRAW[pp, 0], in_=vt3[:, 1:3, wh * 128:(wh + 1) * 128])
        for k in range(NF):
            nc.sync.dma_start(
                out=MRAW[pp, k + 1],
                in_=va4[k, :, 1:3, wh * 128:(wh + 1) * 128])

    # ---- c_feats DMAs, two per frame (one per w-half) ----
    for f in range(F):
        src = cf[:, f].rearrange("c h (wh w) -> wh h c w", wh=2, w=NJ)
        for wh in range(2):
            pp = slice(wh * 64, (wh + 1) * 64)
            nc.sync.dma_start(out=CF[pp, f], in_=src[wh])

    # c0 passthrough to out0[0:128] straight from SBUF
    dst_c0 = out0[0:C].rearrange("c h (wh w) -> wh h c w", wh=2, w=NJ)
    for wh in range(2):
        nc.sync.dma_start(out=dst_c0[wh], in_=CF[wh * 64:(wh + 1) * 64, 0])

    # ---- resize + threshold masks ----
    for k in range(F):
        r0 = MRAW[:, k, 0].rearrange("p (w r4) -> p w r4", r4=4)
        r1 = MRAW[:, k, 1].rearrange("p (w r4) -> p w r4", r4=4)
        t1 = js.tile([P, NJ], F32, tag="mt1")
        t2 = js.tile([P, NJ], F32, tag="mt2")
        nc.vector.tensor_tensor(out=t1[:], in0=r0[:, :, 1], in1=r0[:, :, 2],
                                op=AluOpType.add)
        nc.vector.tensor_tensor(out=t2[:], in0=r1[:, :, 1], in1=r1[:, :, 2],
                                op=AluOpType.add)
        nc.vector.tensor_tensor(out=t1[:], in0=t1[:], in1=t2[:],
                                op=AluOpType.add)
        nc.vector.tensor_scalar(out=VR[:, k], in0=t1[:], scalar1=2.0,
                                scalar2=None, op0=AluOpType.is_gt)

    for f in range(NF):
        nc.vector.tensor_tensor(out=VMAP[:, f], in0=VR[:, 0], in1=VR[:, f + 1],
                                op=AluOpType.mult)

    # ---- A phase: D[p, f, j] = sum_c c0[p,c,j] * crest[p,f,c,j] ----
    for f in range(NF):
        for j in range(NJ):
            c0j = CF[:, 0, :, j]
            crj = CF[:, f + 1, :, j]
            dcol = D[:, f, j:j + 1]
            junk = jv.tile([P, C], F32, tag="jvA")
            nc.vector.scalar_tensor_tensor(
                out=junk[:], in0=c0j, scalar=0.0, in1=crj,
                op0=AluOpType.add, op1=AluOpType.mult, accum_out=dcol)

    # ---- gs: per-partition partials then partition all-reduce ----
    for f in range(NF):
        junk = js.tile([P, NJ], F32, tag="jsA")
        nc.vector.scalar_tensor_tensor(
            out=junk[:], in0=D[:, f], scalar=0.0, in1=VMAP[:, f],
            op0=AluOpType.add, op1=AluOpType.mult, accum_out=GN[:, f:f + 1])
        junk2 = js.tile([P, NJ], F32, tag="jsB")
        nc.vector.scalar_tensor_tensor(
            out=junk2[:], in0=VR[:, 0], scalar=0.0, in1=VR[:, f + 1],
            op0=AluOpType.add, op1=AluOpType.mult,
            accum_out=GN[:, 7 + f:8 + f])

    nc.gpsimd.partition_all_reduce(
        out_ap=GA[:, 0:14], in_ap=GN[:, 0:14], channels=P,
        reduce_op=bass_isa.ReduceOp.add)

    VS = GA[:, 7:14]
    GNUM = GA[:, 0:7]
    # z = vsum < eps ; vsum2 = vsum + z ; gsn = vsum2 * C ; gs = gnum/gsn*(1-z)
    nc.vector.tensor_scalar(out=T7a[:], in0=VS, scalar1=EPS, scalar2=None,
                            op0=AluOpType.is_lt)
    nc.vector.tensor_tensor(out=T7b[:], in0=VS, in1=T7a[:], op=AluOpType.add)
    nc.vector.tensor_scalar(out=T7b[:], in0=T7b[:], scalar1=float(C),
                            scalar2=None, op0=AluOpType.mult)
    nc.vector.reciprocal(out=T7b[:], in_=T7b[:])
    nc.vector.tensor_tensor(out=GS7[:], in0=GNUM, in1=T7b[:], op=AluOpType.mult)
    nc.vector.tensor_scalar(out=T7a[:], in0=T7a[:], scalar1=-1.0, scalar2=1.0,
                            op0=AluOpType.mult, op1=AluOpType.add)
    nc.vector.tensor_tensor(out=GS7[:], in0=GS7[:], in1=T7a[:],
                            op=AluOpType.mult)

    # ---- per-pixel masked softmax over f ----
    for f in range(NF):
        nc.vector.tensor_scalar(out=MV[:, f], in0=VR[:, f + 1],
                                scalar1=GS7[:, f:f + 1], scalar2=None,
                                op0=AluOpType.mult)
    mv_wf = MV[:, :, :].rearrange("p f w -> p w f")
    nc.vector.tensor_reduce(out=MX[:], in_=mv_wf, axis=mybir.AxisListType.X,
                            op=AluOpType.max)
    for f in range(NF):
        nc.vector.tensor_tensor(out=MV[:, f], in0=MV[:, f], in1=MX[:],
                                op=AluOpType.subtract)
        nc.scalar.activation(out=EX[:, f], in_=MV[:, f],
                             func=mybir.ActivationFunctionType.Exp)
        nc.vector.tensor_tensor(out=EX[:, f], in0=EX[:, f], in1=VR[:, f + 1],
                                op=AluOpType.mult)
    ex_wf = EX[:, :, :].rearrange("p f w -> p w f")
    nc.vector.tensor_reduce(out=SM[:], in_=ex_wf, axis=mybir.AxisListType.X,
                            op=AluOpType.add)
    nc.vector.tensor_scalar(out=T32[:], in0=SM[:], scalar1=EPS, scalar2=None,
                            op0=AluOpType.is_lt)
    nc.vector.tensor_tensor(out=SM[:], in0=SM[:], in1=T32[:], op=AluOpType.add)
    nc.vector.reciprocal(out=SM[:], in_=SM[:])
    for f in range(NF):
        nc.vector.tensor_tensor(out=WT[:, f], in0=EX[:, f], in1=SM[:],
                                op=AluOpType.mult)

    # c_mask = 1 - sum_f W
    wt_wf = WT[:, :, :].rearrange("p f w -> p w f")
    nc.vector.tensor_reduce(out=CM[:], in_=wt_wf, axis=mybir.AxisListType.X,
                            op=AluOpType.add)
    nc.vector.tensor_scalar(out=CM[:], in0=CM[:], scalar1=-1.0, scalar2=1.0,
                            op0=AluOpType.mult, op1=AluOpType.add)

    dst_cm0 = out0[C * 2:C * 2 + 1].rearrange("c h (wh w) -> wh h c w",
                                              wh=2, w=NJ)
    dst_cm1 = out1[0:1].rearrange("c h (wh w) -> wh h c w", wh=2, w=NJ)
    cm3 = CM[:].rearrange("p (c w) -> p c w", c=1)
    for wh in range(2):
        pp = slice(wh * 64, (wh + 1) * 64)
        nc.sync.dma_start(out=dst_cm0[wh], in_=cm3[pp])
        nc.sync.dma_start(out=dst_cm1[wh], in_=cm3[pp])

    # ---- B phase: ACC[p, c, j] = sum_f W[p,f,j] * crest[p,f,c,j] ----
    GP_B = True
    for j in range(NJ):
        if j % 2 == 0 or not GP_B:
            for f in range(NF):
                crj = CF[:, f + 1, :, j]
                wcol = WT[:, f, j:j + 1]
                if f == 0:
                    nc.vector.tensor_scalar(
                        out=ACC[:, :, j], in0=crj, scalar1=wcol,
                        scalar2=None, op0=AluOpType.mult)
                else:
                    nc.vector.scalar_tensor_tensor(
                        out=ACC[:, :, j], in0=crj, scalar=wcol,
                        in1=ACC[:, :, j], op0=AluOpType.mult,
                        op1=AluOpType.add)
        else:
            for f in range(NF):
                crj = CF[:, f + 1, :, j]
                wb = WT[:, f, j:j + 1].broadcast_to((P, C))
                if f == 0:
                    nc.gpsimd.tensor_tensor(out=ACC[:, :, j], in0=crj,
                                            in1=wb, op=AluOpType.mult)
                else:
                    junk = jg.tile([P, C], F32, tag="jgB")
                    nc.gpsimd.tensor_tensor(out=junk[:], in0=crj, in1=wb,
                                            op=AluOpType.mult)
                    nc.gpsimd.tensor_tensor(out=ACC[:, :, j],
                                            in0=ACC[:, :, j], in1=junk[:],
                                            op=AluOpType.add)

    dst_cout = out0[C:2 * C].rearrange("c h (wh w) -> wh h c w", wh=2, w=NJ)
    for wh in range(2):
        nc.sync.dma_start(out=dst_cout[wh], in_=ACC[wh * 64:(wh + 1) * 64])


def _build():
    if "nc" in _cache:
        return _cache["nc"]
    nc = bacc.Bacc("TRN2", target_bir_lowering=False, debug=False)
    cf = nc.dram_tensor("cf", [C, F, H, W], F32, kind="ExternalInput").ap()
    vt = nc.dram_tensor("vt", [256, 256], F32, kind="ExternalInput").ap()
    va = nc.dram_tensor("va", [NF, 256, 256], F32, kind="ExternalInput").ap()
    out0 = nc.dram_tensor("out0", [2 * C + 1, H, W], F32,
                          kind="ExternalOutput").ap()
    out1 = nc.dram_tensor("out1", [1, H, W], F32, kind="ExternalOutput").ap()

    with tile.TileContext(nc) as tc:
        with ExitStack() as ctx:
            _emit(tc, ctx, cf, vt, va, out0, out1)
    nc.compile()
    _cache["nc"] = nc
    return nc


def kernel(c_feats, v_t, v_aligned, _trace=False):
    c_feats = np.ascontiguousarray(np.asarray(c_feats, dtype=np.float32))
    v_t = np.ascontiguousarray(np.asarray(v_t, dtype=np.float32))
    v_aligned = np.ascontiguousarray(np.asarray(v_aligned, dtype=np.float32))
    b = c_feats.shape[0]
    assert b == 8

    nc = _build()
    in_maps = [
        {"cf": c_feats[i], "vt": v_t[i, 0], "va": v_aligned[i, 0]}
        for i in range(b)
    ]
    res = run_bass_kernel_spmd(nc, in_maps, list(range(b)), trace=_trace)
    o0 = np.stack([res.results[i]["out0"] for i in range(b)])
    o1 = np.stack([res.results[i]["out1"] for i in range(b)])
    if _trace:
        kernel.last_exec_time_ns = res.exec_time_ns
        kernel.last_results = res
    return o0, o1
